# revision 21
# baseline (speedup 1.0000x reference)
"""Trainium2 Bass kernel for nn_DigitConvolutionalModel (dense CNN -> MLP).

Pure data parallel over 8 NeuronCores (2048 samples each). The 3x3 conv is
linear, so the host folds it into the first FC layer (W1e = C @ w1.T), making
the whole network a 4-layer MLP computed in transposed orientation (features
on partitions, batch on the free dim) in fp16 (psum fp32, ~5e-4 rel err):

    outT = w4t.T @ relu(w3t.T @ relu(w2t.T @ relu(W1e.T @ xT + b1) + b2) + b3) + b4

Raw bass with manual semaphores. DMA pieces are contiguous DRAM tensors
spread across both HWDGE rings in strict need order with balanced bytes
(per-ring throughput is ~135 GB/s; aggregate ~260). The tensor engine opens
with a full-array warmup burst sized to the DMA-bound L1 start (~12.5us) so
the HAM clock-gate reaches 8/8 before real work, and dummy matmuls bridge
the x-tile waits so the PE never re-throttles. The two h1-relu halves run in
parallel on ACT (m0) and DVE (m1).

PE op order (A=L1, B=L2, C=L3, D=L4):
  A0 A1 B0 A2 C0 B1 A3 D0 C1 B2 D1 C2 B3 D2 C3 D3
ACT: r(0,0) r(1,0) r(2,0) h3(0) r(3,0) h3(1) h3(2) h3(3)        (sa +1 each)
DVE: r1(0) r1(1) h2(0) r1(2) h2(1) r1(3) out(0) h2(2) out(1)
     h2(3) out(2) out(3)                                         (sv +1 each)
s2 counts PE tail ops (B/C/D) in PE order.
"""

from contextlib import ExitStack

import ml_dtypes
import numpy as np

import concourse.bass as bass
import concourse.mybir as mybir

N_CORES = 8
B = 16384
BC = B // N_CORES
NB = 512
NT = BC // NB
KC = 112
NKC = 7

F32 = mybir.dt.float32
BF16 = mybir.dt.bfloat16
FP16 = mybir.dt.float16
RELU = mybir.ActivationFunctionType.Relu
ADD = mybir.AluOpType.add
MAX = mybir.AluOpType.max

N_WARM_MM = 15

# t0 in halves for an earlier L1 start; t1-t3 whole (fewer receipt stalls)
X_SPLITS = [[(0, 4), (4, 7)], [(0, 7)], [(0, 7)], [(0, 7)]]
W1_SPLITS = [(0, 4), (4, 7)]

PE_ORDER = [
    ("A", 0), ("A", 1), ("B", 0), ("A", 2), ("C", 0), ("B", 1), ("A", 3),
    ("D", 0), ("C", 1), ("B", 2), ("D", 1), ("C", 2), ("Ba", 3), ("Bb", 3),
    ("D", 2), ("Ca", 3), ("Cb", 3), ("Da", 3), ("Db", 3),
]
TAILS = [(k, t) for (k, t) in PE_ORDER if k != "A"]
POS_PE = {op: i + 1 for i, op in enumerate(TAILS)}  # s2 thresholds

ACT_ORDER = [
    ("r", 0, 0), ("r", 1, 0), ("r", 2, 0), ("h3", 0), ("ra", 3), ("rb", 3),
    ("h3", 1), ("h3", 2), ("h3a", 3), ("h3b", 3),
]
POS_A = {op: i + 1 for i, op in enumerate(ACT_ORDER)}  # sa thresholds

DVE_ORDER = [
    ("r1", 0), ("r1", 1), ("h2", 0), ("r1", 2), ("h2", 1), ("r1a", 3),
    ("r1b", 3), ("out", 0), ("h2", 2), ("out", 1), ("h2a", 3), ("h2b", 3),
    ("out", 2), ("outa", 3), ("outb", 3),
]
POS_V = {op: i + 1 for i, op in enumerate(DVE_ORDER)}  # sv thresholds


def build_program(l1_dt=FP16, l234_dt=FP16):
    nc = bass.Bass()

    n_wp = 256 + 64 + 10

    # One contiguous DRAM tensor per DMA piece.
    xp_d = [
        [
            nc.declare_dram_parameter(
                f"xp{t}_{i}", [KC, (c1 - c0) * NB], l1_dt, isOutput=False
            )
            for i, (c0, c1) in enumerate(X_SPLITS[t])
        ]
        for t in range(NT)
    ]
    w1p_d = [
        nc.declare_dram_parameter(
            f"w1p{i}", [KC, (c1 - c0) * 256], l1_dt, isOutput=False
        )
        for i, (c0, c1) in enumerate(W1_SPLITS)
    ]
    wp_d = nc.declare_dram_parameter("wpack", [128, n_wp], l234_dt, isOutput=False)
    bp_d = nc.declare_dram_parameter("bpack", [128, 5], F32, isOutput=False)
    out_d = nc.declare_dram_parameter("outT", [NT, 10, NB], F32, isOutput=True)

    ctx = ExitStack()
    with ctx:
        xsb = ctx.enter_context(nc.sbuf_tensor([KC, NT, NKC, NB], l1_dt))
        w1sb = ctx.enter_context(nc.sbuf_tensor([KC, NKC, 256], l1_dt))
        wpsb = ctx.enter_context(nc.sbuf_tensor([128, n_wp], l234_dt))
        bpsb = ctx.enter_context(nc.sbuf_tensor([128, 5], F32))
        h1sb = ctx.enter_context(nc.sbuf_tensor([128, 2, 2, NB], l234_dt))
        h2sb = ctx.enter_context(nc.sbuf_tensor([128, 2, NB], l234_dt))
        h3sb = ctx.enter_context(nc.sbuf_tensor([64, 2, NB], l234_dt))
        osb = ctx.enter_context(nc.sbuf_tensor([10, NT, NB], F32))
        warm = ctx.enter_context(nc.sbuf_tensor([1, 513], BF16))
        junk = ctx.enter_context(nc.sbuf_tensor([128, 128 + NB], FP16))
        dump_a = ctx.enter_context(nc.sbuf_tensor([1, 16], BF16))

        w2v = wpsb[:, 0:256].rearrange("p (c o) -> p c o", c=2)
        w3v = wpsb[:, 256:320]
        w4v = wpsb[0:64, 320:330]
        b1v = bpsb[:, 0:2]
        b2v = bpsb[:, 2:3]
        b3v = bpsb[0:64, 3:4]
        b4v = bpsb[0:10, 4:5]

        ps1 = ctx.enter_context(nc.psum_tensor([128, 2, 2, NB], F32))
        ps2 = ctx.enter_context(nc.psum_tensor([128, NB], F32))
        ps3 = ctx.enter_context(nc.psum_tensor([64, NB], F32))
        ps4 = ctx.enter_context(nc.psum_tensor([10, NB], F32))
        psd = ctx.enter_context(nc.psum_tensor([128, NB], F32))

        sx = [
            [ctx.enter_context(nc.semaphore(f"sx{t}_{i}")) for i in range(len(X_SPLITS[t]))]
            for t in range(NT)
        ]
        sw1 = [ctx.enter_context(nc.semaphore(f"sw1_{i}")) for i in range(len(W1_SPLITS))]
        swr = ctx.enter_context(nc.semaphore("swr"))
        sm = ctx.enter_context(nc.semaphore("sm"))
        s2 = ctx.enter_context(nc.semaphore("s2"))
        sa = ctx.enter_context(nc.semaphore("sa"))
        sv = ctx.enter_context(nc.semaphore("sv"))
        sof = ctx.enter_context(nc.semaphore("sof"))

        block = ctx.enter_context(nc.Block())

        @block.sync
        def _(sy):
            # qSP ring, need-ordered: w1a, t0B, w1b, wpack, bpack, t3, outs
            sy.dma_start(out=w1sb[:, 0:4, :], in_=w1p_d[0][:]).then_inc(sw1[0], 16)
            sy.dma_start(out=xsb[:, 0, 4:7, :], in_=xp_d[0][1][:]).then_inc(
                sx[0][1], 16
            )
            sy.dma_start(out=w1sb[:, 4:7, :], in_=w1p_d[1][:]).then_inc(sw1[1], 16)
            sy.dma_start(out=wpsb[:], in_=wp_d[:]).then_inc(swr, 16)
            sy.dma_start(out=bpsb[:], in_=bp_d[:]).then_inc(swr, 16)
            sy.dma_start(out=xsb[:, 3, :, :], in_=xp_d[3][0][:]).then_inc(
                sx[3][0], 16
            )
            for t in range(NT):
                key = ("outb", 3) if t == 3 else ("out", t)
                sy.wait_ge(sv, POS_V[key])
                sy.dma_start(out=out_d[t], in_=osb[:, t, :]).then_inc(sof, 16)
            sy.wait_ge(sof, 16 * NT)

        @block.scalar
        def _(se):
            # qAct ring, need-ordered: t0A, t1, t2
            se.dma_start(out=xsb[:, 0, 0:4, :], in_=xp_d[0][0][:]).then_inc(
                sx[0][0], 16
            )
            se.dma_start(out=xsb[:, 1, :, :], in_=xp_d[1][0][:]).then_inc(
                sx[1][0], 16
            )
            se.dma_start(out=xsb[:, 2, :, :], in_=xp_d[2][0][:]).then_inc(
                sx[2][0], 16
            )
            se.activation(dump_a[:], warm[:, 0:16], RELU)  # preload relu table
            se.wait_ge(swr, 32)
            for op in ACT_ORDER:
                if op[0] == "r":
                    _, t, _m = op
                    st = t % 2
                    if t >= 2:
                        se.wait_ge(s2, POS_PE[("B", t - 2)])  # h1 slot free
                    se.wait_ge(sm, 2 * t + 1)
                    se.activation(
                        h1sb[:, st, 0, :], ps1[:, st, 0, :], RELU,
                        bias=b1v[:, 0:1],
                    ).then_inc(sa, 1)
                elif op[0] in ("ra", "rb"):
                    lo = 0 if op[0] == "ra" else 256
                    if op[0] == "ra":
                        se.wait_ge(s2, POS_PE[("B", 1)])  # h1 slot free
                        se.wait_ge(sm, 7)
                    se.activation(
                        h1sb[:, 1, 0, lo : lo + 256],
                        ps1[:, 1, 0, lo : lo + 256], RELU,
                        bias=b1v[:, 0:1],
                    ).then_inc(sa, 1)
                elif op[0] == "h3":
                    _, t = op
                    st = t % 2
                    se.wait_ge(s2, POS_PE[("C", t)])
                    se.activation(
                        h3sb[:, st, :], ps3[:], RELU, bias=b3v[:]
                    ).then_inc(sa, 1)
                elif op[0] == "h3a":
                    se.wait_ge(s2, POS_PE[("Ca", 3)])
                    se.activation(
                        h3sb[:, 1, 0:256], ps3[:, 0:256], RELU, bias=b3v[:]
                    ).then_inc(sa, 1)
                else:
                    se.wait_ge(s2, POS_PE[("Cb", 3)])
                    se.activation(
                        h3sb[:, 1, 256:512], psd[0:64, 0:256], RELU, bias=b3v[:]
                    ).then_inc(sa, 1)

        @block.vector
        def _(ve):
            ve.wait_ge(swr, 32)
            for op in DVE_ORDER:
                kind, t = op
                st = t % 2
                if kind == "r1":
                    if t >= 2:
                        ve.wait_ge(s2, POS_PE[("B", t - 2)])  # h1 slot free
                    ve.wait_ge(sm, 2 * t + 2)
                    ve.tensor_scalar(
                        h1sb[:, st, 1, :], ps1[:, st, 1, :], b1v[:, 1:2],
                        0.0, ADD, MAX,
                    ).then_inc(sv, 1)
                elif kind in ("r1a", "r1b"):
                    lo = 0 if kind == "r1a" else 256
                    if kind == "r1a":
                        ve.wait_ge(s2, POS_PE[("B", 1)])  # h1 slot free
                        ve.wait_ge(sm, 8)
                    ve.tensor_scalar(
                        h1sb[:, 1, 1, lo : lo + 256],
                        ps1[:, 1, 1, lo : lo + 256], b1v[:, 1:2],
                        0.0, ADD, MAX,
                    ).then_inc(sv, 1)
                elif kind == "h2":
                    ve.wait_ge(s2, POS_PE[("B", t)])
                    ve.tensor_scalar(
                        h2sb[:, st, :], ps2[:], b2v[:], 0.0, ADD, MAX
                    ).then_inc(sv, 1)
                elif kind == "h2a":
                    ve.wait_ge(s2, POS_PE[("Ba", 3)])
                    ve.tensor_scalar(
                        h2sb[:, 1, 0:256], ps2[:, 0:256], b2v[:], 0.0, ADD, MAX
                    ).then_inc(sv, 1)
                elif kind == "h2b":
                    ve.wait_ge(s2, POS_PE[("Bb", 3)])
                    ve.tensor_scalar(
                        h2sb[:, 1, 256:512], ps1[:, 1, 0, 0:256], b2v[:],
                        0.0, ADD, MAX,
                    ).then_inc(sv, 1)
                elif kind == "out":
                    ve.wait_ge(s2, POS_PE[("D", t)])
                    src_ps = ps1[0:10, 0, 1, :] if t == 2 else ps4[:]
                    ve.tensor_scalar(
                        osb[:, t, :], src_ps, b4v[:], None, ADD
                    ).then_inc(sv, 1)
                elif kind == "outa":
                    ve.wait_ge(s2, POS_PE[("Da", 3)])
                    ve.tensor_scalar(
                        osb[:, 3, 0:256], ps4[:, 0:256], b4v[:], None, ADD
                    ).then_inc(sv, 1)
                else:
                    ve.wait_ge(s2, POS_PE[("Db", 3)])
                    ve.tensor_scalar(
                        osb[:, 3, 256:512], psd[0:10, 256:512], b4v[:],
                        None, ADD,
                    ).then_inc(sv, 1)

        @block.tensor
        def _(te):
            # Full-array warmup burst: lifts the HAM clock gate to 8/8 while
            # the first DMAs land. Reads uninitialized SBUF (values
            # irrelevant), dumps into a dedicated psum bank.
            def dummy_mm(k):
                for _i in range(k):
                    te.matmul(psd[:, :], junk[:, 0:128], junk[:, 128:],
                              start=True, stop=True)

            dummy_mm(N_WARM_MM)

            def emit_L1(t):
                st = t % 2
                if t >= 2:
                    te.wait_ge(sa, POS_A[("r", t - 2, 0)])  # ps1 m0 free
                    te.wait_ge(sv, POS_V[("r1", t - 2)])    # ps1 m1 free
                for c in range(NKC):
                    for i, (a, _b) in enumerate(X_SPLITS[t]):
                        if a == c:
                            te.wait_ge(sx[t][i], 16)
                    if t == 0:
                        for i, (a, _b) in enumerate(W1_SPLITS):
                            if a == c:
                                te.wait_ge(sw1[i], 16)
                    for m in range(2):
                        mm = te.matmul(
                            ps1[:, st, m, :],
                            w1sb[:, c, m * 128 : (m + 1) * 128],
                            xsb[:, t, c, :],
                            start=(c == 0),
                            stop=(c == NKC - 1),
                        )
                        if c == NKC - 1:
                            mm.then_inc(sm, 1)

            for kind, t in PE_ORDER:
                st = t % 2
                if kind == "A":
                    if t >= 1:
                        dummy_mm(3)  # warmth insurance
                    emit_L1(t)
                elif kind == "B":
                    if t == 0:
                        te.wait_ge(swr, 32)
                    te.wait_ge(sa, POS_A[("r", t, 0)])
                    if t >= 1:
                        te.wait_ge(sv, POS_V[("h2", t - 1)])  # ps2 free
                    te.matmul(
                        ps2[:], w2v[:, 0, :], h1sb[:, st, 0, :],
                        start=True, stop=False,
                    )
                    te.wait_ge(sv, POS_V[("r1", t)])
                    te.matmul(
                        ps2[:], w2v[:, 1, :], h1sb[:, st, 1, :],
                        start=False, stop=True,
                    ).then_inc(s2, 1)
                elif kind in ("Ba", "Bb"):
                    # half-width L2 chains for the last tile; Bb targets the
                    # freed ps1[st=1,m=0] bank so h2a (DVE read of ps2) can
                    # overlap Bb's PE writes.
                    lo = 0 if kind == "Ba" else 256
                    dst = ps2[:, 0:256] if kind == "Ba" else ps1[:, 1, 0, 0:256]
                    te.wait_ge(sa, POS_A[("ra" if kind == "Ba" else "rb", 3)])
                    te.wait_ge(sv, POS_V[("r1a" if kind == "Ba" else "r1b", 3)])
                    if kind == "Ba":
                        te.wait_ge(sv, POS_V[("h2", 2)])  # ps2 free
                    te.matmul(
                        dst, w2v[:, 0, :], h1sb[:, 1, 0, lo : lo + 256],
                        start=True, stop=False,
                    )
                    te.matmul(
                        dst, w2v[:, 1, :], h1sb[:, 1, 1, lo : lo + 256],
                        start=False, stop=True,
                    ).then_inc(s2, 1)
                elif kind == "C":
                    te.wait_ge(sv, POS_V[("h2", t)])
                    te.matmul(
                        ps3[:], w3v[:], h2sb[:, st, :], start=True, stop=True
                    ).then_inc(s2, 1)
                elif kind in ("Ca", "Cb"):
                    half = "h2a" if kind == "Ca" else "h2b"
                    lo = 0 if kind == "Ca" else 256
                    dst = ps3[:, 0:256] if kind == "Ca" else psd[0:64, 0:256]
                    te.wait_ge(sv, POS_V[(half, 3)])
                    te.matmul(
                        dst, w3v[:], h2sb[:, 1, lo : lo + 256],
                        start=True, stop=True,
                    ).then_inc(s2, 1)
                elif kind == "D":
                    te.wait_ge(sa, POS_A[("h3", t)])
                    if t == 2:
                        # rehome into the freed ps1[st0,m1] bank so Da3 does
                        # not have to wait for out(2)'s read of ps4
                        te.wait_ge(sv, POS_V[("r1", 2)])
                        dst = ps1[0:10, 0, 1, :]
                    else:
                        if t >= 1:
                            te.wait_ge(sv, POS_V[("out", t - 1)])  # ps4 free
                        dst = ps4[:]
                    te.matmul(
                        dst, w4v[:], h3sb[:, st, :], start=True, stop=True
                    ).then_inc(s2, 1)
                elif kind == "Da":
                    te.wait_ge(sa, POS_A[("h3a", 3)])
                    te.wait_ge(sv, POS_V[("out", 1)])  # ps4 free (t2 rehomed)
                    te.matmul(
                        ps4[:, 0:256], w4v[:], h3sb[:, 1, 0:256],
                        start=True, stop=True,
                    ).then_inc(s2, 1)
                else:  # Db
                    te.wait_ge(sa, POS_A[("h3b", 3)])
                    te.matmul(
                        psd[0:10, 256:512], w4v[:], h3sb[:, 1, 256:512],
                        start=True, stop=True,
                    ).then_inc(s2, 1)

    return nc


def _np_dt(dt):
    if dt == BF16:
        return ml_dtypes.bfloat16
    if dt == FP16:
        return np.float16
    return np.float32


def prepare_inputs(x, conv_w, w1, b1, w2, b2, w3, b3, w4, b4,
                   l1_dt=FP16, l234_dt=FP16):
    w1v = np.ascontiguousarray(w1.T).reshape(26, 26, 256)
    w1e = np.zeros((28, 28, 256), dtype=np.float32)
    for di in range(3):
        for dj in range(3):
            w1e[di : di + 26, dj : dj + 26, :] += conv_w[di, dj] * w1v
    w1e = w1e.reshape(784, 256)
    w1t = np.ascontiguousarray(
        w1e.reshape(NKC, KC, 256).transpose(1, 0, 2)
    ).reshape(KC, NKC * 256).astype(_np_dt(l1_dt))
    w1pieces = {}
    for i, (c0, c1) in enumerate(W1_SPLITS):
        w1pieces[f"w1p{i}"] = np.ascontiguousarray(
            w1t.reshape(KC, NKC, 256)[:, c0:c1, :].reshape(KC, (c1 - c0) * 256)
        )

    w2t = np.ascontiguousarray(w2.T).reshape(2, 128, 128).transpose(1, 0, 2)
    wpack = np.zeros((128, 256 + 64 + 10), dtype=np.float32)
    wpack[:, 0:256] = w2t.reshape(128, 256)
    wpack[:, 256:320] = w3.T
    wpack[0:64, 320:330] = w4.T
    wpack = wpack.astype(_np_dt(l234_dt))

    bpack = np.zeros((128, 5), dtype=np.float32)
    bpack[:, 0:2] = b1.reshape(2, 128).T
    bpack[:, 2] = b2
    bpack[0:64, 3] = b3
    bpack[0:10, 4] = b4

    shared = {"wpack": wpack, "bpack": bpack, **w1pieces}
    in_maps = []
    for m in range(N_CORES):
        xc = x[m * BC : (m + 1) * BC]
        xt = np.ascontiguousarray(
            xc.reshape(NT, NB, NKC, KC).transpose(0, 3, 2, 1)
        ).astype(_np_dt(l1_dt))
        d = dict(shared)
        for t in range(NT):
            for i, (c0, c1) in enumerate(X_SPLITS[t]):
                d[f"xp{t}_{i}"] = np.ascontiguousarray(
                    xt[t, :, c0:c1, :].reshape(KC, (c1 - c0) * NB)
                )
        in_maps.append(d)
    return in_maps



_PROGRAM = None


def _get_program():
    global _PROGRAM
    if _PROGRAM is None:
        _PROGRAM = build_program()
    return _PROGRAM


def kernel(x, conv_w, w1, b1, w2, b2, w3, b3, w4, b4):
    from concourse import bass_utils

    args = [x, conv_w, w1, b1, w2, b2, w3, b3, w4, b4]
    x, conv_w, w1, b1, w2, b2, w3, b3, w4, b4 = [
        np.asarray(a, dtype=np.float32) for a in args
    ]
    nc = _get_program()
    in_maps = prepare_inputs(x, conv_w, w1, b1, w2, b2, w3, b3, w4, b4)
    res = bass_utils.run_bass_kernel_spmd(nc, in_maps, list(range(N_CORES)))
    out = np.concatenate(
        [
            res.results[m]["outT"].transpose(0, 2, 1).reshape(BC, 10)
            for m in range(N_CORES)
        ],
        axis=0,
    )
    return out.astype(np.float32)


# revision 22
# speedup vs baseline: 1.1176x; 1.1176x over previous
"""Trainium2 Bass kernel for nn_DigitConvolutionalModel (dense CNN -> MLP).

Pure data parallel over 8 NeuronCores (2048 samples each). The 3x3 conv is
linear, so the host folds it into the first FC layer (W1e = C @ w1.T), making
the whole network a 4-layer MLP computed in transposed orientation (features
on partitions, batch on the free dim) in fp16 (psum fp32, ~5e-4 rel err):

    outT = w4t.T @ relu(w3t.T @ relu(w2t.T @ relu(W1e.T @ xT + b1) + b2) + b3) + b4

Raw bass with manual semaphores. DMA pieces are contiguous DRAM tensors
spread across both HWDGE rings in strict need order with balanced bytes
(per-ring throughput is ~135 GB/s; aggregate ~260). The tensor engine opens
with a full-array warmup burst sized to the DMA-bound L1 start (~12.5us) so
the HAM clock-gate reaches 8/8 before real work, and dummy matmuls bridge
the x-tile waits so the PE never re-throttles. The two h1-relu halves run in
parallel on ACT (m0) and DVE (m1).

PE op order (A=L1, B=L2, C=L3, D=L4):
  A0 A1 B0 A2 C0 B1 A3 D0 C1 B2 D1 C2 B3 D2 C3 D3
ACT: r(0,0) r(1,0) r(2,0) h3(0) r(3,0) h3(1) h3(2) h3(3)        (sa +1 each)
DVE: r1(0) r1(1) h2(0) r1(2) h2(1) r1(3) out(0) h2(2) out(1)
     h2(3) out(2) out(3)                                         (sv +1 each)
s2 counts PE tail ops (B/C/D) in PE order.
"""

from contextlib import ExitStack

import ml_dtypes
import numpy as np

import concourse.bass as bass
import concourse.mybir as mybir

N_CORES = 8
B = 16384
BC = B // N_CORES
NB = 512
NT = BC // NB
KC = 112
NKC = 7

F32 = mybir.dt.float32
BF16 = mybir.dt.bfloat16
FP16 = mybir.dt.float16
RELU = mybir.ActivationFunctionType.Relu
ADD = mybir.AluOpType.add
MAX = mybir.AluOpType.max

N_WARM_MM = 15

# t0 in halves for an earlier L1 start; t1-t3 whole (fewer receipt stalls)
X_SPLITS = [[(0, 4), (4, 7)], [(0, 7)], [(0, 7)], [(0, 7)]]
W1_SPLITS = [(0, 7)]

PE_ORDER = [
    ("A", 0), ("A", 1), ("B", 0), ("A", 2), ("C", 0), ("B", 1), ("A", 3),
    ("D", 0), ("C", 1), ("B", 2), ("D", 1), ("C", 2), ("Ba", 3), ("Bb", 3),
    ("D", 2), ("Ca", 3), ("Cb", 3), ("Da", 3), ("Db", 3),
]
TAILS = [(k, t) for (k, t) in PE_ORDER if k != "A"]
POS_PE = {op: i + 1 for i, op in enumerate(TAILS)}  # s2 thresholds

ACT_ORDER = [
    ("r", 0, 0), ("r", 1, 0), ("r", 2, 0), ("h3", 0), ("ra", 3), ("rb", 3),
    ("h3", 1), ("h3", 2), ("h3a", 3), ("h3b", 3),
]
POS_A = {op: i + 1 for i, op in enumerate(ACT_ORDER)}  # sa thresholds

DVE_ORDER = [
    ("r1", 0), ("r1", 1), ("h2", 0), ("r1", 2), ("h2", 1), ("r1a", 3),
    ("r1b", 3), ("out", 0), ("h2", 2), ("out", 1), ("h2a", 3), ("h2b", 3),
    ("out", 2), ("outa", 3), ("outb", 3),
]
POS_V = {op: i + 1 for i, op in enumerate(DVE_ORDER)}  # sv thresholds


def build_program(l1_dt=FP16, l234_dt=FP16):
    nc = bass.Bass()

    n_wp = 256 + 64 + 10

    # One contiguous DRAM tensor per DMA piece.
    xp_d = [
        [
            nc.declare_dram_parameter(
                f"xp{t}_{i}", [KC, (c1 - c0) * NB], l1_dt, isOutput=False
            )
            for i, (c0, c1) in enumerate(X_SPLITS[t])
        ]
        for t in range(NT)
    ]
    w1p_d = [
        nc.declare_dram_parameter(
            f"w1p{i}", [KC, (c1 - c0) * 256], l1_dt, isOutput=False
        )
        for i, (c0, c1) in enumerate(W1_SPLITS)
    ]
    wp_d = nc.declare_dram_parameter("wpack", [128, n_wp], l234_dt, isOutput=False)
    bp_d = nc.declare_dram_parameter("bpack", [128, 5], F32, isOutput=False)
    out_d = nc.declare_dram_parameter("outT", [NT, 10, NB], F32, isOutput=True)

    ctx = ExitStack()
    with ctx:
        xsb = ctx.enter_context(nc.sbuf_tensor([KC, NT, NKC, NB], l1_dt))
        w1sb = ctx.enter_context(nc.sbuf_tensor([KC, NKC, 256], l1_dt))
        wpsb = ctx.enter_context(nc.sbuf_tensor([128, n_wp], l234_dt))
        bpsb = ctx.enter_context(nc.sbuf_tensor([128, 5], F32))
        h1sb = ctx.enter_context(nc.sbuf_tensor([128, 2, 2, NB], l234_dt))
        h2sb = ctx.enter_context(nc.sbuf_tensor([128, 2, NB], l234_dt))
        h3sb = ctx.enter_context(nc.sbuf_tensor([64, 2, NB], l234_dt))
        osb = ctx.enter_context(nc.sbuf_tensor([10, NT, NB], F32))
        warm = ctx.enter_context(nc.sbuf_tensor([1, 513], BF16))
        junk = ctx.enter_context(nc.sbuf_tensor([128, 128 + NB], FP16))
        dump_a = ctx.enter_context(nc.sbuf_tensor([1, 16], BF16))

        w2v = wpsb[:, 0:256].rearrange("p (c o) -> p c o", c=2)
        w3v = wpsb[:, 256:320]
        w4v = wpsb[0:64, 320:330]
        b1v = bpsb[:, 0:2]
        b2v = bpsb[:, 2:3]
        b3v = bpsb[0:64, 3:4]
        b4v = bpsb[0:10, 4:5]

        ps1 = ctx.enter_context(nc.psum_tensor([128, 2, 2, NB], F32))
        ps2 = ctx.enter_context(nc.psum_tensor([128, NB], F32))
        ps3 = ctx.enter_context(nc.psum_tensor([64, NB], F32))
        ps4 = ctx.enter_context(nc.psum_tensor([10, NB], F32))
        psd = ctx.enter_context(nc.psum_tensor([128, NB], F32))

        sx = [
            [ctx.enter_context(nc.semaphore(f"sx{t}_{i}")) for i in range(len(X_SPLITS[t]))]
            for t in range(NT)
        ]
        sw1 = [ctx.enter_context(nc.semaphore(f"sw1_{i}")) for i in range(len(W1_SPLITS))]
        swr = ctx.enter_context(nc.semaphore("swr"))
        sm = ctx.enter_context(nc.semaphore("sm"))
        s2 = ctx.enter_context(nc.semaphore("s2"))
        sa = ctx.enter_context(nc.semaphore("sa"))
        sv = ctx.enter_context(nc.semaphore("sv"))
        sof = ctx.enter_context(nc.semaphore("sof"))

        block = ctx.enter_context(nc.Block())

        @block.sync
        def _(sy):
            # qSP ring, need-ordered: w1, wpack, bpack, t2, t3, outs
            sy.dma_start(out=w1sb[:, :, :], in_=w1p_d[0][:]).then_inc(sw1[0], 16)
            sy.dma_start(out=wpsb[:], in_=wp_d[:]).then_inc(swr, 16)
            sy.dma_start(out=bpsb[:], in_=bp_d[:]).then_inc(swr, 16)
            sy.dma_start(out=xsb[:, 2, :, :], in_=xp_d[2][0][:]).then_inc(
                sx[2][0], 16
            )
            sy.dma_start(out=xsb[:, 3, :, :], in_=xp_d[3][0][:]).then_inc(
                sx[3][0], 16
            )
            for t in range(NT):
                key = ("outb", 3) if t == 3 else ("out", t)
                sy.wait_ge(sv, POS_V[key])
                sy.dma_start(out=out_d[t], in_=osb[:, t, :]).then_inc(sof, 16)
            sy.wait_ge(sof, 16 * NT)

        @block.scalar
        def _(se):
            # qAct ring, need-ordered: t0A, t0B, t1
            se.dma_start(out=xsb[:, 0, 0:4, :], in_=xp_d[0][0][:]).then_inc(
                sx[0][0], 16
            )
            se.dma_start(out=xsb[:, 0, 4:7, :], in_=xp_d[0][1][:]).then_inc(
                sx[0][1], 16
            )
            se.dma_start(out=xsb[:, 1, :, :], in_=xp_d[1][0][:]).then_inc(
                sx[1][0], 16
            )
            se.activation(dump_a[:], warm[:, 0:16], RELU)  # preload relu table
            se.wait_ge(swr, 32)
            for op in ACT_ORDER:
                if op[0] == "r":
                    _, t, _m = op
                    st = t % 2
                    if t >= 2:
                        se.wait_ge(s2, POS_PE[("B", t - 2)])  # h1 slot free
                    se.wait_ge(sm, 2 * t + 1)
                    se.activation(
                        h1sb[:, st, 0, :], ps1[:, st, 0, :], RELU,
                        bias=b1v[:, 0:1],
                    ).then_inc(sa, 1)
                elif op[0] in ("ra", "rb"):
                    lo = 0 if op[0] == "ra" else 256
                    if op[0] == "ra":
                        se.wait_ge(s2, POS_PE[("B", 1)])  # h1 slot free
                        se.wait_ge(sm, 7)
                    se.activation(
                        h1sb[:, 1, 0, lo : lo + 256],
                        ps1[:, 1, 0, lo : lo + 256], RELU,
                        bias=b1v[:, 0:1],
                    ).then_inc(sa, 1)
                elif op[0] == "h3":
                    _, t = op
                    st = t % 2
                    se.wait_ge(s2, POS_PE[("C", t)])
                    se.activation(
                        h3sb[:, st, :], ps3[:], RELU, bias=b3v[:]
                    ).then_inc(sa, 1)
                elif op[0] == "h3a":
                    se.wait_ge(s2, POS_PE[("Ca", 3)])
                    se.activation(
                        h3sb[:, 1, 0:256], ps3[:, 0:256], RELU, bias=b3v[:]
                    ).then_inc(sa, 1)
                else:
                    se.wait_ge(s2, POS_PE[("Cb", 3)])
                    se.activation(
                        h3sb[:, 1, 256:512], psd[0:64, 0:256], RELU, bias=b3v[:]
                    ).then_inc(sa, 1)

        @block.vector
        def _(ve):
            ve.wait_ge(swr, 32)
            for op in DVE_ORDER:
                kind, t = op
                st = t % 2
                if kind == "r1":
                    if t >= 2:
                        ve.wait_ge(s2, POS_PE[("B", t - 2)])  # h1 slot free
                    ve.wait_ge(sm, 2 * t + 2)
                    ve.tensor_scalar(
                        h1sb[:, st, 1, :], ps1[:, st, 1, :], b1v[:, 1:2],
                        0.0, ADD, MAX,
                    ).then_inc(sv, 1)
                elif kind in ("r1a", "r1b"):
                    lo = 0 if kind == "r1a" else 256
                    if kind == "r1a":
                        ve.wait_ge(s2, POS_PE[("B", 1)])  # h1 slot free
                        ve.wait_ge(sm, 8)
                    ve.tensor_scalar(
                        h1sb[:, 1, 1, lo : lo + 256],
                        ps1[:, 1, 1, lo : lo + 256], b1v[:, 1:2],
                        0.0, ADD, MAX,
                    ).then_inc(sv, 1)
                elif kind == "h2":
                    ve.wait_ge(s2, POS_PE[("B", t)])
                    ve.tensor_scalar(
                        h2sb[:, st, :], ps2[:], b2v[:], 0.0, ADD, MAX
                    ).then_inc(sv, 1)
                elif kind == "h2a":
                    ve.wait_ge(s2, POS_PE[("Ba", 3)])
                    ve.tensor_scalar(
                        h2sb[:, 1, 0:256], ps2[:, 0:256], b2v[:], 0.0, ADD, MAX
                    ).then_inc(sv, 1)
                elif kind == "h2b":
                    ve.wait_ge(s2, POS_PE[("Bb", 3)])
                    ve.tensor_scalar(
                        h2sb[:, 1, 256:512], ps1[:, 1, 0, 0:256], b2v[:],
                        0.0, ADD, MAX,
                    ).then_inc(sv, 1)
                elif kind == "out":
                    ve.wait_ge(s2, POS_PE[("D", t)])
                    src_ps = ps1[0:10, 0, 1, :] if t == 2 else ps4[:]
                    ve.tensor_scalar(
                        osb[:, t, :], src_ps, b4v[:], None, ADD
                    ).then_inc(sv, 1)
                elif kind == "outa":
                    ve.wait_ge(s2, POS_PE[("Da", 3)])
                    ve.tensor_scalar(
                        osb[:, 3, 0:256], ps4[:, 0:256], b4v[:], None, ADD
                    ).then_inc(sv, 1)
                else:
                    ve.wait_ge(s2, POS_PE[("Db", 3)])
                    ve.tensor_scalar(
                        osb[:, 3, 256:512], psd[0:10, 256:512], b4v[:],
                        None, ADD,
                    ).then_inc(sv, 1)

        @block.tensor
        def _(te):
            # Full-array warmup burst: lifts the HAM clock gate to 8/8 while
            # the first DMAs land. Reads uninitialized SBUF (values
            # irrelevant), dumps into a dedicated psum bank.
            def dummy_mm(k):
                for _i in range(k):
                    te.matmul(psd[:, :], junk[:, 0:128], junk[:, 128:],
                              start=True, stop=True)

            dummy_mm(N_WARM_MM)

            def emit_L1(t):
                st = t % 2
                if t >= 2:
                    te.wait_ge(sa, POS_A[("r", t - 2, 0)])  # ps1 m0 free
                    te.wait_ge(sv, POS_V[("r1", t - 2)])    # ps1 m1 free
                for c in range(NKC):
                    for i, (a, _b) in enumerate(X_SPLITS[t]):
                        if a == c:
                            te.wait_ge(sx[t][i], 16)
                    if t == 0:
                        for i, (a, _b) in enumerate(W1_SPLITS):
                            if a == c:
                                te.wait_ge(sw1[i], 16)
                    for m in range(2):
                        mm = te.matmul(
                            ps1[:, st, m, :],
                            w1sb[:, c, m * 128 : (m + 1) * 128],
                            xsb[:, t, c, :],
                            start=(c == 0),
                            stop=(c == NKC - 1),
                        )
                        if c == NKC - 1:
                            mm.then_inc(sm, 1)

            for kind, t in PE_ORDER:
                st = t % 2
                if kind == "A":
                    if t >= 1:
                        dummy_mm(3)  # warmth insurance
                    emit_L1(t)
                elif kind == "B":
                    if t == 0:
                        te.wait_ge(swr, 32)
                    te.wait_ge(sa, POS_A[("r", t, 0)])
                    if t >= 1:
                        te.wait_ge(sv, POS_V[("h2", t - 1)])  # ps2 free
                    te.matmul(
                        ps2[:], w2v[:, 0, :], h1sb[:, st, 0, :],
                        start=True, stop=False,
                    )
                    te.wait_ge(sv, POS_V[("r1", t)])
                    te.matmul(
                        ps2[:], w2v[:, 1, :], h1sb[:, st, 1, :],
                        start=False, stop=True,
                    ).then_inc(s2, 1)
                elif kind in ("Ba", "Bb"):
                    # half-width L2 chains for the last tile; Bb targets the
                    # freed ps1[st=1,m=0] bank so h2a (DVE read of ps2) can
                    # overlap Bb's PE writes.
                    lo = 0 if kind == "Ba" else 256
                    dst = ps2[:, 0:256] if kind == "Ba" else ps1[:, 1, 0, 0:256]
                    te.wait_ge(sa, POS_A[("ra" if kind == "Ba" else "rb", 3)])
                    te.wait_ge(sv, POS_V[("r1a" if kind == "Ba" else "r1b", 3)])
                    if kind == "Ba":
                        te.wait_ge(sv, POS_V[("h2", 2)])  # ps2 free
                    te.matmul(
                        dst, w2v[:, 0, :], h1sb[:, 1, 0, lo : lo + 256],
                        start=True, stop=False,
                    )
                    te.matmul(
                        dst, w2v[:, 1, :], h1sb[:, 1, 1, lo : lo + 256],
                        start=False, stop=True,
                    ).then_inc(s2, 1)
                elif kind == "C":
                    te.wait_ge(sv, POS_V[("h2", t)])
                    te.matmul(
                        ps3[:], w3v[:], h2sb[:, st, :], start=True, stop=True
                    ).then_inc(s2, 1)
                elif kind in ("Ca", "Cb"):
                    half = "h2a" if kind == "Ca" else "h2b"
                    lo = 0 if kind == "Ca" else 256
                    dst = ps3[:, 0:256] if kind == "Ca" else psd[0:64, 0:256]
                    te.wait_ge(sv, POS_V[(half, 3)])
                    te.matmul(
                        dst, w3v[:], h2sb[:, 1, lo : lo + 256],
                        start=True, stop=True,
                    ).then_inc(s2, 1)
                elif kind == "D":
                    te.wait_ge(sa, POS_A[("h3", t)])
                    if t == 2:
                        # rehome into the freed ps1[st0,m1] bank so Da3 does
                        # not have to wait for out(2)'s read of ps4
                        te.wait_ge(sv, POS_V[("r1", 2)])
                        dst = ps1[0:10, 0, 1, :]
                    else:
                        if t >= 1:
                            te.wait_ge(sv, POS_V[("out", t - 1)])  # ps4 free
                        dst = ps4[:]
                    te.matmul(
                        dst, w4v[:], h3sb[:, st, :], start=True, stop=True
                    ).then_inc(s2, 1)
                elif kind == "Da":
                    te.wait_ge(sa, POS_A[("h3a", 3)])
                    te.wait_ge(sv, POS_V[("out", 1)])  # ps4 free (t2 rehomed)
                    te.matmul(
                        ps4[:, 0:256], w4v[:], h3sb[:, 1, 0:256],
                        start=True, stop=True,
                    ).then_inc(s2, 1)
                else:  # Db
                    te.wait_ge(sa, POS_A[("h3b", 3)])
                    te.matmul(
                        psd[0:10, 256:512], w4v[:], h3sb[:, 1, 256:512],
                        start=True, stop=True,
                    ).then_inc(s2, 1)

    return nc


def _np_dt(dt):
    if dt == BF16:
        return ml_dtypes.bfloat16
    if dt == FP16:
        return np.float16
    return np.float32


def prepare_inputs(x, conv_w, w1, b1, w2, b2, w3, b3, w4, b4,
                   l1_dt=FP16, l234_dt=FP16):
    w1v = np.ascontiguousarray(w1.T).reshape(26, 26, 256)
    w1e = np.zeros((28, 28, 256), dtype=np.float32)
    for di in range(3):
        for dj in range(3):
            w1e[di : di + 26, dj : dj + 26, :] += conv_w[di, dj] * w1v
    w1e = w1e.reshape(784, 256)
    w1t = np.ascontiguousarray(
        w1e.reshape(NKC, KC, 256).transpose(1, 0, 2)
    ).reshape(KC, NKC * 256).astype(_np_dt(l1_dt))
    w1pieces = {}
    for i, (c0, c1) in enumerate(W1_SPLITS):
        w1pieces[f"w1p{i}"] = np.ascontiguousarray(
            w1t.reshape(KC, NKC, 256)[:, c0:c1, :].reshape(KC, (c1 - c0) * 256)
        )

    w2t = np.ascontiguousarray(w2.T).reshape(2, 128, 128).transpose(1, 0, 2)
    wpack = np.zeros((128, 256 + 64 + 10), dtype=np.float32)
    wpack[:, 0:256] = w2t.reshape(128, 256)
    wpack[:, 256:320] = w3.T
    wpack[0:64, 320:330] = w4.T
    wpack = wpack.astype(_np_dt(l234_dt))

    bpack = np.zeros((128, 5), dtype=np.float32)
    bpack[:, 0:2] = b1.reshape(2, 128).T
    bpack[:, 2] = b2
    bpack[0:64, 3] = b3
    bpack[0:10, 4] = b4

    shared = {"wpack": wpack, "bpack": bpack, **w1pieces}
    in_maps = []
    for m in range(N_CORES):
        xc = x[m * BC : (m + 1) * BC]
        xt = np.ascontiguousarray(
            xc.reshape(NT, NB, NKC, KC).transpose(0, 3, 2, 1)
        ).astype(_np_dt(l1_dt))
        d = dict(shared)
        for t in range(NT):
            for i, (c0, c1) in enumerate(X_SPLITS[t]):
                d[f"xp{t}_{i}"] = np.ascontiguousarray(
                    xt[t, :, c0:c1, :].reshape(KC, (c1 - c0) * NB)
                )
        in_maps.append(d)
    return in_maps



_PROGRAM = None


def _get_program():
    global _PROGRAM
    if _PROGRAM is None:
        _PROGRAM = build_program()
    return _PROGRAM


def kernel(x, conv_w, w1, b1, w2, b2, w3, b3, w4, b4):
    from concourse import bass_utils

    args = [x, conv_w, w1, b1, w2, b2, w3, b3, w4, b4]
    x, conv_w, w1, b1, w2, b2, w3, b3, w4, b4 = [
        np.asarray(a, dtype=np.float32) for a in args
    ]
    nc = _get_program()
    in_maps = prepare_inputs(x, conv_w, w1, b1, w2, b2, w3, b3, w4, b4)
    res = bass_utils.run_bass_kernel_spmd(nc, in_maps, list(range(N_CORES)))
    out = np.concatenate(
        [
            res.results[m]["outT"].transpose(0, 2, 1).reshape(BC, 10)
            for m in range(N_CORES)
        ],
        axis=0,
    )
    return out.astype(np.float32)


# revision 23
# speedup vs baseline: 1.1982x; 1.0721x over previous
"""Trainium2 Bass kernel for nn_DigitConvolutionalModel (dense CNN -> MLP).

Pure data parallel over 8 NeuronCores (2048 samples each). The 3x3 conv is
linear, so the host folds it into the first FC layer (W1e = C @ w1.T), making
the whole network a 4-layer MLP computed in transposed orientation (features
on partitions, batch on the free dim) in fp16 (psum fp32, ~5e-4 rel err):

    outT = w4t.T @ relu(w3t.T @ relu(w2t.T @ relu(W1e.T @ xT + b1) + b2) + b3) + b4

Raw bass with manual semaphores. DMA pieces are contiguous DRAM tensors
spread across both HWDGE rings in strict need order with balanced bytes
(per-ring throughput is ~135 GB/s; aggregate ~260). The tensor engine opens
with a full-array warmup burst sized to the DMA-bound L1 start (~12.5us) so
the HAM clock-gate reaches 8/8 before real work, and dummy matmuls bridge
the x-tile waits so the PE never re-throttles. The two h1-relu halves run in
parallel on ACT (m0) and DVE (m1).

PE op order (A=L1, B=L2, C=L3, D=L4):
  A0 A1 B0 A2 C0 B1 A3 D0 C1 B2 D1 C2 B3 D2 C3 D3
ACT: r(0,0) r(1,0) r(2,0) h3(0) r(3,0) h3(1) h3(2) h3(3)        (sa +1 each)
DVE: r1(0) r1(1) h2(0) r1(2) h2(1) r1(3) out(0) h2(2) out(1)
     h2(3) out(2) out(3)                                         (sv +1 each)
s2 counts PE tail ops (B/C/D) in PE order.
"""

from contextlib import ExitStack

import ml_dtypes
import numpy as np

import concourse.bass as bass
import concourse.mybir as mybir

N_CORES = 8
B = 16384
BC = B // N_CORES
NB = 512
NT = BC // NB
KC = 112
NKC = 7

F32 = mybir.dt.float32
BF16 = mybir.dt.bfloat16
FP16 = mybir.dt.float16
RELU = mybir.ActivationFunctionType.Relu
ADD = mybir.AluOpType.add
MAX = mybir.AluOpType.max

N_WARM_MM = 15

# t0 in halves for an earlier L1 start; t1-t3 whole (fewer receipt stalls)
X_SPLITS = [[(0, 4), (4, 7)], [(0, 7)], [(0, 7)], [(0, 7)]]
W1_SPLITS = [(0, 7)]

PE_ORDER = [
    ("A", 0), ("A", 1), ("B", 0), ("A", 2), ("C", 0), ("B", 1), ("A", 3),
    ("D", 0), ("C", 1), ("B", 2), ("D", 1), ("C", 2), ("Ba", 3), ("Bb", 3),
    ("D", 2), ("Ca", 3), ("Cb", 3), ("Da", 3), ("Db", 3),
]
TAILS = [(k, t) for (k, t) in PE_ORDER if k != "A"]
POS_PE = {op: i + 1 for i, op in enumerate(TAILS)}  # s2 thresholds

ACT_ORDER = [
    ("r", 0, 0), ("r", 1, 0), ("r", 2, 0), ("h3", 0), ("ra", 3), ("rb", 3),
    ("h3", 1), ("h3", 2), ("h3a", 3), ("h3b", 3),
]
POS_A = {op: i + 1 for i, op in enumerate(ACT_ORDER)}  # sa thresholds

DVE_ORDER = [
    ("r1", 0), ("r1", 1), ("h2", 0), ("r1", 2), ("h2", 1), ("r1a", 3),
    ("r1b", 3), ("out", 0), ("h2", 2), ("out", 1), ("h2a", 3), ("h2b", 3),
    ("out", 2), ("outa", 3), ("outb", 3),
]
POS_V = {op: i + 1 for i, op in enumerate(DVE_ORDER)}  # sv thresholds


def build_program(l1_dt=FP16, l234_dt=FP16):
    nc = bass.Bass()

    n_wp = 256 + 64 + 10

    # One contiguous DRAM tensor per DMA piece.
    xp_d = [
        [
            nc.declare_dram_parameter(
                f"xp{t}_{i}", [KC, (c1 - c0) * NB], l1_dt, isOutput=False
            )
            for i, (c0, c1) in enumerate(X_SPLITS[t])
        ]
        for t in range(NT)
    ]
    w1p_d = [
        nc.declare_dram_parameter(
            f"w1p{i}", [KC, (c1 - c0) * 256], l1_dt, isOutput=False
        )
        for i, (c0, c1) in enumerate(W1_SPLITS)
    ]
    wp_d = nc.declare_dram_parameter("wpack", [128, n_wp], l234_dt, isOutput=False)
    bp_d = nc.declare_dram_parameter("bpack", [128, 5], F32, isOutput=False)
    out_d = nc.declare_dram_parameter("outT", [NT, 10, NB], F32, isOutput=True)

    ctx = ExitStack()
    with ctx:
        xsb = ctx.enter_context(nc.sbuf_tensor([KC, NT, NKC, NB], l1_dt))
        w1sb = ctx.enter_context(nc.sbuf_tensor([KC, NKC, 256], l1_dt))
        wpsb = ctx.enter_context(nc.sbuf_tensor([128, n_wp], l234_dt))
        bpsb = ctx.enter_context(nc.sbuf_tensor([128, 5], F32))
        h1sb = ctx.enter_context(nc.sbuf_tensor([128, 2, 2, NB], l234_dt))
        h2sb = ctx.enter_context(nc.sbuf_tensor([128, 2, NB], l234_dt))
        h3sb = ctx.enter_context(nc.sbuf_tensor([64, 2, NB], l234_dt))
        osb = ctx.enter_context(nc.sbuf_tensor([10, NT, NB], F32))
        warm = ctx.enter_context(nc.sbuf_tensor([1, 513], BF16))
        junk = ctx.enter_context(nc.sbuf_tensor([128, 128 + NB], FP16))
        dump_a = ctx.enter_context(nc.sbuf_tensor([1, 16], BF16))

        w2v = wpsb[:, 0:256].rearrange("p (c o) -> p c o", c=2)
        w3v = wpsb[:, 256:320]
        w4v = wpsb[0:64, 320:330]
        b1v = bpsb[:, 0:2]
        b2v = bpsb[:, 2:3]
        b3v = bpsb[0:64, 3:4]
        b4v = bpsb[0:10, 4:5]

        ps1 = ctx.enter_context(nc.psum_tensor([128, 2, 2, NB], F32))
        ps2 = ctx.enter_context(nc.psum_tensor([128, NB], F32))
        ps3 = ctx.enter_context(nc.psum_tensor([64, NB], F32))
        ps4 = ctx.enter_context(nc.psum_tensor([10, NB], F32))
        psd = ctx.enter_context(nc.psum_tensor([128, NB], F32))

        sx = [
            [ctx.enter_context(nc.semaphore(f"sx{t}_{i}")) for i in range(len(X_SPLITS[t]))]
            for t in range(NT)
        ]
        sw1 = [ctx.enter_context(nc.semaphore(f"sw1_{i}")) for i in range(len(W1_SPLITS))]
        swr = ctx.enter_context(nc.semaphore("swr"))
        sm = ctx.enter_context(nc.semaphore("sm"))
        s2 = ctx.enter_context(nc.semaphore("s2"))
        sa = ctx.enter_context(nc.semaphore("sa"))
        sv = ctx.enter_context(nc.semaphore("sv"))
        sof = ctx.enter_context(nc.semaphore("sof"))

        block = ctx.enter_context(nc.Block())

        @block.sync
        def _(sy):
            # qSP ring, need-ordered: w1, wpack, bpack, t2, t3, outs
            sy.dma_start(out=w1sb[:, :, :], in_=w1p_d[0][:]).then_inc(sw1[0], 16)
            sy.dma_start(out=wpsb[:], in_=wp_d[:]).then_inc(swr, 16)
            sy.dma_start(out=bpsb[:], in_=bp_d[:]).then_inc(swr, 16)
            sy.dma_start(out=xsb[:, 2, :, :], in_=xp_d[2][0][:]).then_inc(
                sx[2][0], 16
            )
            sy.dma_start(out=xsb[:, 3, :, :], in_=xp_d[3][0][:]).then_inc(
                sx[3][0], 16
            )
            for t in range(NT):
                key = ("outb", 3) if t == 3 else ("out", t)
                sy.wait_ge(sv, POS_V[key])
                sy.dma_start(out=out_d[t], in_=osb[:, t, :]).then_inc(sof, 16)
            sy.wait_ge(sof, 16 * NT)

        @block.scalar
        def _(se):
            # qAct ring, need-ordered: t0A, t0B, t1
            se.dma_start(out=xsb[:, 0, 0:4, :], in_=xp_d[0][0][:]).then_inc(
                sx[0][0], 16
            )
            se.dma_start(out=xsb[:, 0, 4:7, :], in_=xp_d[0][1][:]).then_inc(
                sx[0][1], 16
            )
            se.dma_start(out=xsb[:, 1, :, :], in_=xp_d[1][0][:]).then_inc(
                sx[1][0], 16
            )
            se.activation(dump_a[:], warm[:, 0:16], RELU)  # preload relu table
            se.wait_ge(swr, 32)
            for op in ACT_ORDER:
                if op[0] == "r":
                    _, t, _m = op
                    st = t % 2
                    if t >= 2:
                        se.wait_ge(s2, POS_PE[("B", t - 2)])  # h1 slot free
                    se.wait_ge(sm, 2 * t + 1)
                    se.activation(
                        h1sb[:, st, 0, :], ps1[:, st, 0, :], RELU,
                        bias=b1v[:, 0:1],
                    ).then_inc(sa, 1)
                elif op[0] in ("ra", "rb"):
                    lo = 0 if op[0] == "ra" else 256
                    if op[0] == "ra":
                        se.wait_ge(s2, POS_PE[("B", 1)])  # h1 slot free
                        se.wait_ge(sm, 7)
                    se.activation(
                        h1sb[:, 1, 0, lo : lo + 256],
                        ps1[:, 1, 0, lo : lo + 256], RELU,
                        bias=b1v[:, 0:1],
                    ).then_inc(sa, 1)
                elif op[0] == "h3":
                    _, t = op
                    st = t % 2
                    se.wait_ge(s2, POS_PE[("C", t)])
                    se.activation(
                        h3sb[:, st, :], ps3[:], RELU, bias=b3v[:]
                    ).then_inc(sa, 1)
                elif op[0] == "h3a":
                    se.wait_ge(s2, POS_PE[("Ca", 3)])
                    se.activation(
                        h3sb[:, 1, 0:256], ps3[:, 0:256], RELU, bias=b3v[:]
                    ).then_inc(sa, 1)
                else:
                    se.wait_ge(s2, POS_PE[("Cb", 3)])
                    se.activation(
                        h3sb[:, 1, 256:512], psd[0:64, 0:256], RELU, bias=b3v[:]
                    ).then_inc(sa, 1)

        @block.vector
        def _(ve):
            ve.wait_ge(swr, 32)
            for op in DVE_ORDER:
                kind, t = op
                st = t % 2
                if kind == "r1":
                    if t >= 2:
                        ve.wait_ge(s2, POS_PE[("B", t - 2)])  # h1 slot free
                    ve.wait_ge(sm, 2 * t + 2)
                    ve.tensor_scalar(
                        h1sb[:, st, 1, :], ps1[:, st, 1, :], b1v[:, 1:2],
                        0.0, ADD, MAX,
                    ).then_inc(sv, 1)
                elif kind in ("r1a", "r1b"):
                    lo = 0 if kind == "r1a" else 256
                    if kind == "r1a":
                        ve.wait_ge(s2, POS_PE[("B", 1)])  # h1 slot free
                        ve.wait_ge(sm, 8)
                    ve.tensor_scalar(
                        h1sb[:, 1, 1, lo : lo + 256],
                        ps1[:, 1, 1, lo : lo + 256], b1v[:, 1:2],
                        0.0, ADD, MAX,
                    ).then_inc(sv, 1)
                elif kind == "h2":
                    ve.wait_ge(s2, POS_PE[("B", t)])
                    ve.tensor_scalar(
                        h2sb[:, st, :], ps2[:], b2v[:], 0.0, ADD, MAX
                    ).then_inc(sv, 1)
                elif kind == "h2a":
                    ve.wait_ge(s2, POS_PE[("Ba", 3)])
                    ve.tensor_scalar(
                        h2sb[:, 1, 0:256], ps2[:, 0:256], b2v[:], 0.0, ADD, MAX
                    ).then_inc(sv, 1)
                elif kind == "h2b":
                    ve.wait_ge(s2, POS_PE[("Bb", 3)])
                    ve.tensor_scalar(
                        h2sb[:, 1, 256:512], ps1[:, 1, 0, 0:256], b2v[:],
                        0.0, ADD, MAX,
                    ).then_inc(sv, 1)
                elif kind == "out":
                    ve.wait_ge(s2, POS_PE[("D", t)])
                    src_ps = ps1[0:10, 0, 1, :] if t == 2 else ps4[:]
                    ve.tensor_scalar(
                        osb[:, t, :], src_ps, b4v[:], None, ADD
                    ).then_inc(sv, 1)
                elif kind == "outa":
                    ve.wait_ge(s2, POS_PE[("Da", 3)])
                    ve.tensor_scalar(
                        osb[:, 3, 0:256], ps4[:, 0:256], b4v[:], None, ADD
                    ).then_inc(sv, 1)
                else:
                    ve.wait_ge(s2, POS_PE[("Db", 3)])
                    ve.tensor_scalar(
                        osb[:, 3, 256:512], psd[0:10, 256:512], b4v[:],
                        None, ADD,
                    ).then_inc(sv, 1)

        @block.tensor
        def _(te):
            # Full-array warmup burst: lifts the HAM clock gate to 8/8 while
            # the first DMAs land. Reads uninitialized SBUF (values
            # irrelevant), dumps into a dedicated psum bank.
            def dummy_mm(k):
                for _i in range(k):
                    te.matmul(psd[:, :], junk[:, 0:128], junk[:, 128:],
                              start=True, stop=True)

            dummy_mm(N_WARM_MM)

            def emit_L1(t):
                st = t % 2
                if t >= 2:
                    te.wait_ge(sa, POS_A[("r", t - 2, 0)])  # ps1 m0 free
                    te.wait_ge(sv, POS_V[("r1", t - 2)])    # ps1 m1 free
                for c in range(NKC):
                    for i, (a, _b) in enumerate(X_SPLITS[t]):
                        if a == c:
                            te.wait_ge(sx[t][i], 16)
                    if t == 0:
                        for i, (a, _b) in enumerate(W1_SPLITS):
                            if a == c:
                                te.wait_ge(sw1[i], 16)
                    for m in range(2):
                        mm = te.matmul(
                            ps1[:, st, m, :],
                            w1sb[:, c, m * 128 : (m + 1) * 128],
                            xsb[:, t, c, :],
                            start=(c == 0),
                            stop=(c == NKC - 1),
                        )
                        if c == NKC - 1:
                            mm.then_inc(sm, 1)

            for kind, t in PE_ORDER:
                st = t % 2
                if kind == "A":
                    if t >= 1:
                        # Warmth staircase: the x-piece sem rises 0->16 as
                        # the 16 SDMA engines finish, so interleave dummy
                        # matmuls with partial-completion waits to keep the
                        # PE busy through a slow arrival (costs 4 dummies
                        # when the tile is already resident).
                        dummy_mm(1)
                        for thr in (4, 8, 12):
                            te.wait_ge(sx[t][0], thr)
                            dummy_mm(1)
                    emit_L1(t)
                elif kind == "B":
                    if t == 0:
                        te.wait_ge(swr, 32)
                    te.wait_ge(sa, POS_A[("r", t, 0)])
                    if t >= 1:
                        te.wait_ge(sv, POS_V[("h2", t - 1)])  # ps2 free
                    te.matmul(
                        ps2[:], w2v[:, 0, :], h1sb[:, st, 0, :],
                        start=True, stop=False,
                    )
                    te.wait_ge(sv, POS_V[("r1", t)])
                    te.matmul(
                        ps2[:], w2v[:, 1, :], h1sb[:, st, 1, :],
                        start=False, stop=True,
                    ).then_inc(s2, 1)
                elif kind in ("Ba", "Bb"):
                    # half-width L2 chains for the last tile; Bb targets the
                    # freed ps1[st=1,m=0] bank so h2a (DVE read of ps2) can
                    # overlap Bb's PE writes.
                    lo = 0 if kind == "Ba" else 256
                    dst = ps2[:, 0:256] if kind == "Ba" else ps1[:, 1, 0, 0:256]
                    te.wait_ge(sa, POS_A[("ra" if kind == "Ba" else "rb", 3)])
                    te.wait_ge(sv, POS_V[("r1a" if kind == "Ba" else "r1b", 3)])
                    if kind == "Ba":
                        te.wait_ge(sv, POS_V[("h2", 2)])  # ps2 free
                    te.matmul(
                        dst, w2v[:, 0, :], h1sb[:, 1, 0, lo : lo + 256],
                        start=True, stop=False,
                    )
                    te.matmul(
                        dst, w2v[:, 1, :], h1sb[:, 1, 1, lo : lo + 256],
                        start=False, stop=True,
                    ).then_inc(s2, 1)
                elif kind == "C":
                    te.wait_ge(sv, POS_V[("h2", t)])
                    te.matmul(
                        ps3[:], w3v[:], h2sb[:, st, :], start=True, stop=True
                    ).then_inc(s2, 1)
                elif kind in ("Ca", "Cb"):
                    half = "h2a" if kind == "Ca" else "h2b"
                    lo = 0 if kind == "Ca" else 256
                    dst = ps3[:, 0:256] if kind == "Ca" else psd[0:64, 0:256]
                    te.wait_ge(sv, POS_V[(half, 3)])
                    te.matmul(
                        dst, w3v[:], h2sb[:, 1, lo : lo + 256],
                        start=True, stop=True,
                    ).then_inc(s2, 1)
                elif kind == "D":
                    te.wait_ge(sa, POS_A[("h3", t)])
                    if t == 2:
                        # rehome into the freed ps1[st0,m1] bank so Da3 does
                        # not have to wait for out(2)'s read of ps4
                        te.wait_ge(sv, POS_V[("r1", 2)])
                        dst = ps1[0:10, 0, 1, :]
                    else:
                        if t >= 1:
                            te.wait_ge(sv, POS_V[("out", t - 1)])  # ps4 free
                        dst = ps4[:]
                    te.matmul(
                        dst, w4v[:], h3sb[:, st, :], start=True, stop=True
                    ).then_inc(s2, 1)
                elif kind == "Da":
                    te.wait_ge(sa, POS_A[("h3a", 3)])
                    te.wait_ge(sv, POS_V[("out", 1)])  # ps4 free (t2 rehomed)
                    te.matmul(
                        ps4[:, 0:256], w4v[:], h3sb[:, 1, 0:256],
                        start=True, stop=True,
                    ).then_inc(s2, 1)
                else:  # Db
                    te.wait_ge(sa, POS_A[("h3b", 3)])
                    te.matmul(
                        psd[0:10, 256:512], w4v[:], h3sb[:, 1, 256:512],
                        start=True, stop=True,
                    ).then_inc(s2, 1)

    return nc


def _np_dt(dt):
    if dt == BF16:
        return ml_dtypes.bfloat16
    if dt == FP16:
        return np.float16
    return np.float32


def prepare_inputs(x, conv_w, w1, b1, w2, b2, w3, b3, w4, b4,
                   l1_dt=FP16, l234_dt=FP16):
    w1v = np.ascontiguousarray(w1.T).reshape(26, 26, 256)
    w1e = np.zeros((28, 28, 256), dtype=np.float32)
    for di in range(3):
        for dj in range(3):
            w1e[di : di + 26, dj : dj + 26, :] += conv_w[di, dj] * w1v
    w1e = w1e.reshape(784, 256)
    w1t = np.ascontiguousarray(
        w1e.reshape(NKC, KC, 256).transpose(1, 0, 2)
    ).reshape(KC, NKC * 256).astype(_np_dt(l1_dt))
    w1pieces = {}
    for i, (c0, c1) in enumerate(W1_SPLITS):
        w1pieces[f"w1p{i}"] = np.ascontiguousarray(
            w1t.reshape(KC, NKC, 256)[:, c0:c1, :].reshape(KC, (c1 - c0) * 256)
        )

    w2t = np.ascontiguousarray(w2.T).reshape(2, 128, 128).transpose(1, 0, 2)
    wpack = np.zeros((128, 256 + 64 + 10), dtype=np.float32)
    wpack[:, 0:256] = w2t.reshape(128, 256)
    wpack[:, 256:320] = w3.T
    wpack[0:64, 320:330] = w4.T
    wpack = wpack.astype(_np_dt(l234_dt))

    bpack = np.zeros((128, 5), dtype=np.float32)
    bpack[:, 0:2] = b1.reshape(2, 128).T
    bpack[:, 2] = b2
    bpack[0:64, 3] = b3
    bpack[0:10, 4] = b4

    shared = {"wpack": wpack, "bpack": bpack, **w1pieces}
    in_maps = []
    for m in range(N_CORES):
        xc = x[m * BC : (m + 1) * BC]
        xt = np.ascontiguousarray(
            xc.reshape(NT, NB, NKC, KC).transpose(0, 3, 2, 1)
        ).astype(_np_dt(l1_dt))
        d = dict(shared)
        for t in range(NT):
            for i, (c0, c1) in enumerate(X_SPLITS[t]):
                d[f"xp{t}_{i}"] = np.ascontiguousarray(
                    xt[t, :, c0:c1, :].reshape(KC, (c1 - c0) * NB)
                )
        in_maps.append(d)
    return in_maps



_PROGRAM = None


def _get_program():
    global _PROGRAM
    if _PROGRAM is None:
        _PROGRAM = build_program()
    return _PROGRAM


def kernel(x, conv_w, w1, b1, w2, b2, w3, b3, w4, b4):
    from concourse import bass_utils

    args = [x, conv_w, w1, b1, w2, b2, w3, b3, w4, b4]
    x, conv_w, w1, b1, w2, b2, w3, b3, w4, b4 = [
        np.asarray(a, dtype=np.float32) for a in args
    ]
    nc = _get_program()
    in_maps = prepare_inputs(x, conv_w, w1, b1, w2, b2, w3, b3, w4, b4)
    res = bass_utils.run_bass_kernel_spmd(nc, in_maps, list(range(N_CORES)))
    out = np.concatenate(
        [
            res.results[m]["outT"].transpose(0, 2, 1).reshape(BC, 10)
            for m in range(N_CORES)
        ],
        axis=0,
    )
    return out.astype(np.float32)
